# revision 30
# baseline (speedup 1.0000x reference)
"""Tensor-parallel GQA attention block for 8 Trainium2 NeuronCores.

Sharding: 32 q-heads / 8 kv-heads split across 8 cores (4 q-heads + 1
kv-head each).  Each core projects q/k/v from the full x, applies RoPE,
runs causal attention for its heads, then the per-core head outputs are
AllGathered (one gather per 512-query s-tile, so the collectives overlap
attention) and every core computes a distinct 256-column slice of the
final wo projection.  Host concatenates the slices.

Key structure (v2):
  * Attention processes heads in PAIRS.  The even head's k/q live on
    partitions 0..63, the odd head's on 64..127, so the two score
    matmuls of a key chunk run CONCURRENTLY on disjoint PE row-groups
    (tile_position row tiling) -- K=64 contraction no longer wastes
    half the array's issue slots.
  * All big inputs are host-prepacked into SBUF-layout ([128, ...]
    p-major) tensors so each resident tile loads with one wide DMA
    (16-32KB per partition line) at full HBM bandwidth.
  * ACT only runs the softmax exps, one instruction per key chunk
    covering both heads of the pair.  In late s-tiles ACT is the
    per-chunk critical path, so independent PE work (next tile's
    projections, wo chunks, kv fixups) is woven between chunks via a
    debt-driven filler queue to keep the PE dense.
  * Softmax is unnormalized; the denominator comes out of the attn@v
    matmul via a ones column and is divided out on DVE.
  * A tiny dummy AllGather during warmup absorbs the ~10us first-CC
    setup cost so the real per-tile gathers start promptly.

PSUM (8 banks): score pairs [128,2,512]f32 x2bufs = 4, attn@v accum
[65,512]f32 x2 (even/odd head) = 2, projection accum x1 = 1,
warmup/fixups/wo x1 = 1.
"""

import sys

sys.path.insert(0, "/opt/trn_rl_repo")

import numpy as np
import ml_dtypes
from contextlib import ExitStack

import concourse.bass as bass
import concourse.tile as tile
from concourse import bacc, mybir
from concourse.bass import ds
from concourse.bass_utils import run_bass_kernel_spmd

BF16 = ml_dtypes.bfloat16
F32 = mybir.dt.float32
BF = mybir.dt.bfloat16

N_CORES = 8
S = 2048          # sequence length
D = 2048          # model dim
DH = 64           # head dim
HPC = 4           # q heads per core
THETA = 10000.0
ST = 512          # s-tile (free dim) size
NT = S // ST      # 4 s-tiles
DK = D // 128     # 16 contraction chunks
OC = HPC * DH     # 256 head-output columns per core

SWAP_MASK = [i ^ 1 for i in range(32)]   # partition p <-> p^1, per quadrant

_CACHE = {}
LAST_RESULT = None


def _build_program():
    nc = bacc.Bacc("TRN2", target_bir_lowering=False, debug=False,
                   num_devices=N_CORES)

    def din(name, shape, dt):
        return nc.dram_tensor(name, shape, dt, kind="ExternalInput")

    # host-prepacked SBUF layouts: one wide DMA per resident tile
    xsb_d = din("xsb", [128, NT * DK * ST], BF)    # [p][t][ko][s]
    wq_d = din("wqsb", [128, DK * OC], BF)         # [p][ko][m]
    wkv_d = din("wkvsb", [128, DK * 128], BF)      # [p][ko][v(64) k(64)]
    wo_d = din("wosb", [128, DK * OC], BF)         # [p][oc][m]
    cosq_d = din("cosq", [128, S], BF)
    sinq_d = din("sinq", [128, S], BF)
    cosk_d = din("cosk", [128, S], BF)    # k tables live in rows 64..127
    sink_d = din("sink", [128, S], BF)
    tri_d = din("tri", [128, 128], BF)
    idn_d = din("ident", [128, 128], BF)

    y_d = nc.dram_tensor("y", [S, OC], F32, kind="ExternalOutput")
    # gather granularity balances the ~13us per-collective latency floor
    # (collectives serialize) against readiness: full-tile gathers for
    # tiles 0/1 (lots of downstream slack), per-head-pair for tiles 2/3
    # so the tail pieces start the moment their pair finishes.
    og_h = [nc.dram_tensor(f"og{t}", [OC, ST], BF) for t in range(2)]
    oga_h = [nc.dram_tensor(f"og_all{t}", [N_CORES * OC, ST], BF,
                            addr_space="Shared") for t in range(2)]
    og_p = [[nc.dram_tensor(f"og{t}_{j}", [128, ST], BF) for j in range(2)]
            for t in range(2, NT)]
    oga_p = [[nc.dram_tensor(f"og_all{t}_{j}", [N_CORES * 128, ST], BF,
                             addr_space="Shared") for j in range(2)]
             for t in range(2, NT)]
    ccw_d = nc.dram_tensor("ccw", [1, 64], BF)
    ccwa_d = nc.dram_tensor("ccw_all", [N_CORES, 64], BF, addr_space="Shared")

    EXP = mybir.ActivationFunctionType.Exp
    RG = [list(range(N_CORES))]

    with tile.TileContext(nc) as tc:
        with ExitStack() as ctx:
            cp = ctx.enter_context(tc.tile_pool(name="const", bufs=1))
            scp = ctx.enter_context(tc.tile_pool(name="scp", bufs=2, space="PSUM"))
            pavp = ctx.enter_context(tc.tile_pool(name="pavp", bufs=1, space="PSUM"))
            prjp = ctx.enter_context(tc.tile_pool(name="prj", bufs=1, space="PSUM"))
            mscp = ctx.enter_context(tc.tile_pool(name="msc", bufs=1, space="PSUM"))
            rawp = ctx.enter_context(tc.tile_pool(name="raw", bufs=4))
            tmpp = ctx.enter_context(tc.tile_pool(name="tmp", bufs=3))
            expp = ctx.enter_context(tc.tile_pool(name="expp", bufs=3))
            nrmp = ctx.enter_context(tc.tile_pool(name="nrm", bufs=4))

            scr_d = nc.dram_tensor("scratch", [128, 8], F32)

            # ---- PE warm-up: keep the HAM activity monitor busy while the
            # input DMAs stream in, so real matmuls start at 2.4 GHz.
            wsrc = cp.tile([128, 512], BF)
            nc.vector.memset(wsrc[:], 0.25)
            # tiny exp primes the ACT table set during warmup (the first
            # real softmax exp would otherwise pay the ~2.7us table load)
            wex = rawp.tile([1, 16], BF, tag="wex")
            nc.scalar.activation(wex[:], wsrc[0:1, 0:16],
                                 mybir.ActivationFunctionType.Exp)
            pw = mscp.tile([128, 512], F32, tag="msc")
            NWARM = 28
            for i in range(NWARM):
                nc.tensor.matmul(pw[:], wsrc[:, 0:128], wsrc[:],
                                 start=(i == 0), stop=(i == NWARM - 1),
                                 skip_group_check=True)
            wout = rawp.tile([128, 8], F32, tag="wout")
            nc.vector.tensor_copy(wout[:], pw[:, 0:8])
            nc.sync.dma_start(scr_d.ap(), wout[:])

            # ---- resident tensors (load order = consumption order) ----
            wkv_sb = cp.tile([128, DK, 128], BF)
            nc.sync.dma_start(wkv_sb[:], wkv_d.ap())
            bigs = [cp.tile([128, DK, ST], BF, name=f"big{i}", tag=f"big{i}")
                    for i in range(NT)]
            for q in range(4):      # quarter-split so proj starts early
                nc.sync.dma_start(bigs[0][:, ds(4 * q, 4), :],
                                  xsb_d[:, ds(4 * q * ST, 4 * ST)])
            cosk = cp.tile([128, S], BF); nc.sync.dma_start(cosk[:], cosk_d.ap())
            sink = cp.tile([128, S], BF); nc.sync.dma_start(sink[:], sink_d.ap())
            tri = cp.tile([128, 128], BF); nc.sync.dma_start(tri[:], tri_d.ap())
            idn = cp.tile([128, 128], BF); nc.sync.dma_start(idn[:], idn_d.ap())
            wq_sb = cp.tile([128, DK, OC], BF)
            nc.sync.dma_start(wq_sb[:], wq_d.ap())
            cosq = cp.tile([128, S], BF); nc.sync.dma_start(cosq[:], cosq_d.ap())
            sinq = cp.tile([128, S], BF); nc.sync.dma_start(sinq[:], sinq_d.ap())
            for t in range(1, NT):
                nc.sync.dma_start(bigs[t][:], xsb_d[:, ds(t * DK * ST, DK * ST)])
            wo_sb = cp.tile([128, DK, OC], BF)
            nc.sync.dma_start(wo_sb[:], wo_d.ap())

            qT = cp.tile([128, 2, S], BF)
            kT2 = cp.tile([128, S], BF)
            vaug = cp.tile([128, DK, DH + 1], BF)
            nc.vector.memset(vaug[:, :, DH:DH + 1], 1.0)

            # ---- phase 1: projections + RoPE, split into filler parts.
            state = {}

            def emit_accum_part(t, j, part, nparts=4):
                X = bigs[t]
                per = DK // nparts
                if part == 0:
                    state[(t, j, "ps")] = prjp.tile([128, ST], F32, tag="proj",
                                                    name=f"ps{t}_{j}")
                ps = state[(t, j, "ps")]
                for d in range(per * part, per * (part + 1)):
                    lhsT = wq_sb[:, d, ds(128 * j, 128)] if j < 2 else wkv_sb[:, d, :]
                    nc.tensor.matmul(ps[:], lhsT, X[:, d, :],
                                     start=(d == 0), stop=(d == DK - 1),
                                     skip_group_check=True)
                if part == nparts - 1:
                    ps = state.pop((t, j, "ps"))
                    raw = rawp.tile([128, ST], BF, tag="raw")
                    nc.vector.tensor_copy(raw[:], ps[:])
                    state[(t, j)] = raw

            def emit_accum(t, j):
                for part in range(4):
                    emit_accum_part(t, j, part)

            def emit_post(t, j):
                raw = state[(t, j)]
                sl = ds(t * ST, ST)
                sw = tmpp.tile([128, ST], BF, tag="sw")
                nc.vector.stream_shuffle(sw[:], raw[:], SWAP_MASK)
                if j < 2:
                    t1 = tmpp.tile([128, ST], BF, tag="tmp")
                    nc.vector.tensor_mul(t1[:], raw[:], cosq[:, sl])
                    t2 = tmpp.tile([128, ST], BF, tag="tmp")
                    nc.vector.tensor_mul(t2[:], sw[:], sinq[:, sl])
                    nc.vector.tensor_add(qT[:, j, sl], t1[:], t2[:])
                    state.pop((t, j))
                else:
                    t1 = tmpp.tile([128, ST], BF, tag="tmp")
                    nc.vector.tensor_mul(t1[64:128], raw[64:128], cosk[64:128, sl])
                    t2 = tmpp.tile([128, ST], BF, tag="tmp")
                    nc.vector.tensor_mul(t2[64:128], sw[64:128], sink[64:128, sl])
                    nc.vector.tensor_add(kT2[64:128, sl], t1[64:128], t2[64:128])

            def emit_post_pe(t, j):
                # PE fixups for the kv projection: duplicate rotated k to
                # partitions 0..63, transpose v into [sk, dh] layout.
                raw = state.pop((t, j))
                sl = ds(t * ST, ST)
                psd = mscp.tile([64, ST], F32, tag="msc")
                nc.tensor.matmul(psd[:], idn[64:128, 64:128], kT2[64:128, sl],
                                 start=True, stop=True)
                nc.vector.tensor_copy(kT2[0:64, sl], psd[:])
                for j4 in range(4):
                    pv = mscp.tile([128, DH], BF, tag="msc")
                    nc.tensor.transpose(pv[:], raw[0:64, ds(128 * j4, 128)],
                                        idn[0:64, 0:64])
                    nc.vector.tensor_copy(vaug[:, 4 * t + j4, 0:DH], pv[:])

            # ---- phase 3: one 128-query chunk of the output projection,
            # split into filler parts accumulating in the msc or prj bank.
            def emit_wo_part(qt, sb, part, alt=0, nparts=4):
                X = bigs[qt]
                pool, tag = ((mscp, "msc"), (prjp, "proj"))[alt % 2]
                key = (qt, sb, "wo")
                if part == 0:
                    state[key] = pool.tile([128, OC], F32, tag=tag,
                                           name=f"wo{qt}_{sb}")
                py = state[key]
                per = DK // nparts
                for oc in range(per * part, per * (part + 1)):
                    nc.tensor.matmul(py[:], X[:, oc, ds(128 * sb, 128)],
                                     wo_sb[:, oc, :],
                                     start=(oc == 0), stop=(oc == DK - 1),
                                     skip_group_check=True)
                if part == nparts - 1:
                    py = state.pop(key)
                    ysb = nrmp.tile([128, OC], F32, tag="yo")
                    nc.vector.tensor_copy(ysb[:], py[:])
                    nc.scalar.dma_start(y_d[ds(qt * ST + sb * 128, 128), :],
                                        ysb[:])

            def emit_wo_chunk(qt, sb, alt=0):
                for part in range(4):
                    emit_wo_part(qt, sb, part, alt=alt)

            # ---- filler queue: independent PE work woven between attention
            # chunks.  Each entry is (cost_ns, closure).
            fillq = []

            def fill(cost, fn):
                fillq.append((cost, fn))

            def pop_fill(budget_ns):
                spent = 0.0
                while fillq and fillq[0][0] <= budget_ns - spent:
                    cost, fn = fillq.pop(0)
                    fn()
                    spent += cost
                return spent

            def drain_fill():
                while fillq:
                    _, fn = fillq.pop(0)
                    fn()

            # ---- attention on s-tile t, head pair j (heads 2j, 2j+1).
            # Per key chunk: two row-tiled score matmuls (concurrent), one
            # ACT exp over both heads, two attn@v accumulation matmuls.
            # ACT is the per-chunk critical path in late tiles, so filler
            # work is popped between chunks to keep the PE dense.
            ACT_NS = {}  # chunk width -> exp ns

            def emit_attn_pair(t, j):
                sl = ds(t * ST, ST)
                nchunk = 4 * t + 4
                pavs = [pavp.tile([DH + 1, ST], F32, tag=f"pav{e}",
                                  name=f"pav{t}_{j}_{e}")
                        for e in (0, 1)]

                def sc_chunk(kc):
                    sc = scp.tile([128, 2, ST], F32, tag="sc")
                    c = kc - 4 * t
                    for e in (0, 1):
                        po = 64 * e
                        if c < 0:
                            nc.tensor.matmul(sc[:, e, :],
                                             kT2[po:po + 64, ds(128 * kc, 128)],
                                             qT[po:po + 64, j, sl],
                                             start=True, stop=True)
                        else:
                            w = ST - 128 * c
                            nc.tensor.matmul(sc[:, e, ds(128 * c, w)],
                                             kT2[po:po + 64, ds(128 * kc, 128)],
                                             qT[po:po + 64, j,
                                                ds(t * ST + 128 * c, w)],
                                             start=True, stop=True)
                    return sc

                def exp_chunk(kc, sc):
                    et = expp.tile([128, 2, ST], BF, tag="et")
                    c = kc - 4 * t
                    if c < 0:
                        nc.scalar.activation(et[:, :, :], sc[:, :, :], EXP)
                    else:
                        w = ST - 128 * c
                        nc.scalar.activation(et[:, :, ds(128 * c, w)],
                                             sc[:, :, ds(128 * c, w)], EXP)
                        for e in (0, 1):
                            nc.vector.tensor_mul(et[:, e, ds(128 * c, 128)],
                                                 et[:, e, ds(128 * c, 128)],
                                                 tri[:])
                    return et

                def av_chunk(kc, et):
                    c = kc - 4 * t
                    first = (kc == 0)
                    last = (kc == nchunk - 1)
                    for e in (0, 1):
                        if c < 0:
                            nc.tensor.matmul(pavs[e][:, :], vaug[:, kc, :],
                                             et[:, e, :],
                                             start=first, stop=last,
                                             skip_group_check=True)
                        else:
                            w = ST - 128 * c
                            nc.tensor.matmul(pavs[e][:, ds(128 * c, w)],
                                             vaug[:, kc, :],
                                             et[:, e, ds(128 * c, w)],
                                             start=first, stop=last,
                                             skip_group_check=True)

                prev = None
                debt = 0.0
                for kc in range(nchunk):
                    sc = sc_chunk(kc)
                    if prev is not None:
                        av_chunk(kc - 1, prev)
                    prev = exp_chunk(kc, sc)
                    c = kc - 4 * t
                    w = ST if c < 0 else ST - 128 * c
                    # ACT exp time minus PE chunk time (PE modeled at the
                    # GPIO-throttled ~2.0 GHz effective clock)
                    debt += (2 * w + 352) / 1.2 - 3 * w / 2.0
                    debt -= pop_fill(debt)
                av_chunk(nchunk - 1, prev)

                # normalization: evacuate the accumulators to SBUF first
                # (frees both PSUM banks for the next pair's attn@v), then
                # reciprocal of the ones-column denominator, broadcast to
                # the 64 head dims, divide, store to og.
                ev = []
                for e in (0, 1):
                    # dn copy remaps the denominator row to partition 0
                    # (reciprocal_approx_fast can't remap partition bases)
                    dn = nrmp.tile([1, ST], F32, tag="dn")
                    nc.vector.tensor_copy(dn[:], pavs[e][DH:DH + 1, :])
                    pvs = nrmp.tile([DH, ST], F32, tag="pvs")
                    nc.vector.tensor_copy(pvs[:], pavs[e][0:DH, :])
                    ev.append((dn, pvs))
                for e in (0, 1):
                    h = 2 * j + e
                    dn, pvs = ev[e]
                    rec = nrmp.tile([1, ST], F32, tag="rec")
                    nc.vector.reciprocal_approx_fast(out=rec[:], in_=dn[:])
                    rep = nrmp.tile([DH, ST], F32, tag="rep")
                    nc.gpsimd.partition_broadcast(rep[:], rec[:])
                    on = nrmp.tile([DH, ST], BF, tag="on")
                    nc.vector.tensor_mul(on[:], pvs[:], rep[:])
                    if t < 2:
                        nc.gpsimd.dma_start(og_h[t][ds(DH * h, DH), :], on[:])
                    else:
                        nc.gpsimd.dma_start(og_p[t - 2][j][ds(DH * e, DH), :],
                                            on[:])

            # ---- prologue: proj(t0), kv first so attention can start sooner
            emit_accum(0, 2)
            emit_accum(0, 0)
            emit_post(0, 2)
            emit_post_pe(0, 2)
            emit_accum(0, 1)
            emit_post(0, 0)
            emit_post(0, 1)

            # ---- main loop over s-tiles.  Filler distribution: next tile's
            # projections weave into the current tile's pairs (at t=0 only
            # into pair 1 -- the bigs[1] input DMA hasn't landed earlier);
            # wo chunks for gathered tiles weave into t=2 pair 1 and t=3.
            for t in range(NT):
                nxt = t + 1 < NT
                for j in range(2):
                    # t=0: no proj fillers -- the bigs[1] input DMA lands
                    # only around the end of tile 0's attention; blocking
                    # filler matmuls would poison the attention pipeline.
                    if nxt and t > 0:
                        if j == 0:
                            for p in range(8):
                                fill(430, lambda t=t, p=p:
                                     emit_accum_part(t + 1, 2, p, nparts=8))
                            for p in range(8):
                                fill(430, lambda t=t, p=p:
                                     emit_accum_part(t + 1, 0, p, nparts=8))
                        else:
                            for p in range(8):
                                fill(430, lambda t=t, p=p:
                                     emit_accum_part(t + 1, 1, p, nparts=8))
                    if t == 2 and j == 1:
                        # wo for gathered tile 0 (its reload landed long ago)
                        for sb in range(2):
                            for p in range(4):
                                fill(450, lambda sb=sb, p=p:
                                     emit_wo_part(0, sb, p, alt=0))
                    if t == 3:
                        # no reload-dependent fillers in tile 3 -- a filler
                        # waiting on a gather reload poisons the last pairs'
                        # attention (the PE FIFO stalls the score stream)
                        # and delays the final gathers.  Pad matmuls keep
                        # the HAM clock warm through the ACT-bound chunks.
                        def pad_fill(tg=f"p3{j}"):
                            pp = mscp.tile([128, 512], F32, tag="msc",
                                           name="padf")
                            for i in range(3):
                                nc.tensor.matmul(pp[:], wsrc[:, 0:128],
                                                 wsrc[:],
                                                 start=(i == 0), stop=(i == 2),
                                                 skip_group_check=True)
                        for _ in range(8):
                            fill(700, pad_fill)
                    emit_attn_pair(t, j)
                    # gather this tile's head outputs; the gathered og
                    # replaces xT in bigs[t] for the wo phase.  Tiles 2/3
                    # gather per pair (pair j's heads land in o-chunks 2c+j).
                    if t >= 2:
                        nc.gpsimd.collective_compute(
                            "AllGather", mybir.AluOpType.bypass,
                            replica_groups=RG,
                            ins=[og_p[t - 2][j].ap()],
                            outs=[oga_p[t - 2][j].ap()])
                        for g in range(2):
                            nc.sync.dma_start(
                                bigs[t][:, ds(8 * g + j, 4, 2), :],
                                oga_p[t - 2][j].ap().rearrange(
                                    "(ko p) m -> p ko m", p=128)[:, ds(4 * g, 4), :])
                    elif j == 1:
                        nc.gpsimd.collective_compute(
                            "AllGather", mybir.AluOpType.bypass,
                            replica_groups=RG,
                            ins=[og_h[t].ap()], outs=[oga_h[t].ap()])
                        for g in range(4):
                            nc.sync.dma_start(
                                bigs[t][:, ds(4 * g, 4), :],
                                oga_h[t].ap().rearrange(
                                    "(ko p) m -> p ko m", p=128)[:, ds(4 * g, 4), :])
                    if nxt and t > 0 and j == 0:
                        drain_fill()
                        emit_post(t + 1, 2)
                        fill(1070, lambda t=t: emit_post_pe(t + 1, 2))
                    if nxt and j == 1:
                        drain_fill()
                        if t == 0:
                            # tile-1 projections emitted whole once the
                            # bigs[1] input DMA has landed
                            emit_accum(1, 2)
                            emit_accum(1, 0)
                            emit_post(1, 2)
                            emit_post_pe(1, 2)
                            emit_accum(1, 1)
                        emit_post(t + 1, 0)
                        emit_post(t + 1, 1)
                    if t == 3:
                        drain_fill()

            # ---- wo for the rest of gathered tile 2, then the last s-tile:
            # even o-chunks (first head pair, landed early) accumulate into
            # partial sums while the second gather is in flight; odd chunks
            # finish after it lands.
            for qt in (0, 1, 2):
                for sb in range(4):
                    if qt == 0 and sb < 2:
                        continue      # woven into tile-2 pair 1
                    emit_wo_chunk(qt, sb, alt=sb)
            yev = []
            for sb in range(4):
                pool, tag = ((mscp, "msc"), (prjp, "proj"))[sb % 2]
                py = pool.tile([128, OC], F32, tag=tag)
                for i, oc in enumerate(range(0, DK, 2)):
                    nc.tensor.matmul(py[:], bigs[3][:, oc, ds(128 * sb, 128)],
                                     wo_sb[:, oc, :],
                                     start=(i == 0), stop=(i == 7))
                ye = nrmp.tile([128, OC], F32, tag="ye")
                nc.vector.tensor_copy(ye[:], py[:])
                yev.append(ye)
            # keep the PE's activity monitor busy while waiting for the
            # second half-gather, so the final matmuls run at 2.4 GHz
            wpad = mscp.tile([128, 512], F32, tag="msc")
            NPAD = 24
            for i in range(NPAD):
                nc.tensor.matmul(wpad[:], wsrc[:, 0:128], wsrc[:],
                                 start=(i == 0), stop=(i == NPAD - 1),
                                 skip_group_check=True)
            for sb in range(4):
                pool, tag = ((mscp, "msc"), (prjp, "proj"))[sb % 2]
                py = pool.tile([128, OC], F32, tag=tag)
                for i, oc in enumerate(range(1, DK, 2)):
                    nc.tensor.matmul(py[:], bigs[3][:, oc, ds(128 * sb, 128)],
                                     wo_sb[:, oc, :],
                                     start=(i == 0), stop=(i == 7))
                ysb = nrmp.tile([128, OC], F32, tag="yo2")
                nc.vector.tensor_add(ysb[:], py[:], yev[sb][:])
                nc.scalar.dma_start(y_d[ds(3 * ST + sb * 128, 128), :], ysb[:])

    nc.compile()
    return nc


def _host_prep(x, wq, wk, wv, wo, pos):
    x2 = np.ascontiguousarray(np.asarray(x).reshape(S, D)).astype(BF16)
    # [p][t][ko][s] p-major pack: one wide DMA per s-tile
    xsb = np.ascontiguousarray(
        x2.reshape(NT, ST, DK, 128).transpose(3, 0, 2, 1).reshape(128, -1))

    posf = np.asarray(pos).astype(np.float32)
    fr = (1.0 / (np.float32(THETA) **
                 (np.arange(0, DH, 2, dtype=np.float32) / np.float32(DH))))
    pf = posf[:, None] * fr[None, :]              # [S, 32] f32
    cos = np.cos(pf).astype(np.float32)
    sin = np.sin(pf).astype(np.float32)
    pidx = np.arange(128)
    fi = (pidx % DH) // 2
    sign = np.where(pidx % 2 == 0, np.float32(-1.0), np.float32(1.0))
    cosq = np.ascontiguousarray(cos[:, fi].T)                  # [128, S]
    sinq = np.ascontiguousarray((sin[:, fi] * sign[None, :]).T)
    kscale = np.float32(1.0 / np.sqrt(DH))
    cosk = np.zeros((128, S), np.float32)
    sink = np.zeros((128, S), np.float32)
    cosk[64:128] = cosq[0:64] * kscale
    sink[64:128] = sinq[0:64] * kscale
    cosq = cosq.astype(BF16); sinq = sinq.astype(BF16)
    cosk = cosk.astype(BF16); sink = sink.astype(BF16)

    tri = np.triu(np.ones((128, 128), np.float32)).astype(BF16)
    idn = np.eye(128, dtype=np.float32).astype(BF16)

    woT = np.asarray(wo).T                        # [o, d]
    in_maps = []
    for c in range(N_CORES):
        wq_c = np.asarray(wq)[OC * c: OC * (c + 1), :].astype(BF16)   # [256, D]
        k_c = np.asarray(wk)[DH * c: DH * (c + 1), :].astype(BF16)    # [64, D]
        v_c = np.asarray(wv)[DH * c: DH * (c + 1), :].astype(BF16)
        wkv_c = np.concatenate([v_c, k_c], axis=0)            # [v, k] [128, D]
        wo_c = np.ascontiguousarray(
            woT[:, OC * c: OC * (c + 1)]).astype(BF16)        # [D, OC]
        in_maps.append({
            "xsb": xsb,
            "wqsb": np.ascontiguousarray(
                wq_c.reshape(OC, DK, 128).transpose(2, 1, 0).reshape(128, -1)),
            "wkvsb": np.ascontiguousarray(
                wkv_c.reshape(128, DK, 128).transpose(2, 1, 0).reshape(128, -1)),
            "wosb": np.ascontiguousarray(
                wo_c.reshape(DK, 128, OC).transpose(1, 0, 2).reshape(128, -1)),
            "cosq": cosq, "sinq": sinq, "cosk": cosk, "sink": sink,
            "tri": tri, "ident": idn,
        })
    return in_maps


def kernel(x, pos, wq, wk, wv, wo):
    global LAST_RESULT
    if "nc" not in _CACHE:
        _CACHE["nc"] = _build_program()
    nc = _CACHE["nc"]
    in_maps = _host_prep(x, wq, wk, wv, wo, pos)
    res = run_bass_kernel_spmd(nc, in_maps, core_ids=list(range(N_CORES)))
    LAST_RESULT = res
    y = np.concatenate([res.results[c]["y"] for c in range(N_CORES)], axis=1)
    return y.reshape(1, S, D).astype(np.float32)


# revision 33
# speedup vs baseline: 1.1858x; 1.1858x over previous
"""Tensor-parallel GQA attention block for 8 Trainium2 NeuronCores.

Sharding: 32 q-heads / 8 kv-heads split across 8 cores (4 q-heads + 1
kv-head each).  Each core projects q/k/v from the full x, applies RoPE,
runs causal attention for its heads, then the per-core head outputs are
AllGathered (one gather per 512-query s-tile, so the collectives overlap
attention) and every core computes a distinct 256-column slice of the
final wo projection.  Host concatenates the slices.

Key structure (v2):
  * Attention processes heads in PAIRS.  The even head's k/q live on
    partitions 0..63, the odd head's on 64..127, so the two score
    matmuls of a key chunk run CONCURRENTLY on disjoint PE row-groups
    (tile_position row tiling) -- K=64 contraction no longer wastes
    half the array's issue slots.
  * All big inputs are host-prepacked into SBUF-layout ([128, ...]
    p-major) tensors so each resident tile loads with one wide DMA
    (16-32KB per partition line) at full HBM bandwidth.
  * ACT only runs the softmax exps, one instruction per key chunk
    covering both heads of the pair.  In late s-tiles ACT is the
    per-chunk critical path, so independent PE work (next tile's
    projections, wo chunks, kv fixups) is woven between chunks via a
    debt-driven filler queue to keep the PE dense.
  * Softmax is unnormalized; the denominator comes out of the attn@v
    matmul via a ones column and is divided out on DVE.
  * A tiny dummy AllGather during warmup absorbs the ~10us first-CC
    setup cost so the real per-tile gathers start promptly.

PSUM (8 banks): score pairs [128,2,512]f32 x2bufs = 4, attn@v accum
[65,512]f32 x2 (even/odd head) = 2, projection accum x1 = 1,
warmup/fixups/wo x1 = 1.
"""

import sys

sys.path.insert(0, "/opt/trn_rl_repo")

import numpy as np
import ml_dtypes
from contextlib import ExitStack

import concourse.bass as bass
import concourse.tile as tile
from concourse import bacc, mybir
from concourse.bass import ds
from concourse.bass_utils import run_bass_kernel_spmd

BF16 = ml_dtypes.bfloat16
F32 = mybir.dt.float32
BF = mybir.dt.bfloat16

N_CORES = 8
S = 2048          # sequence length
D = 2048          # model dim
DH = 64           # head dim
HPC = 4           # q heads per core
THETA = 10000.0
ST = 512          # s-tile (free dim) size
NT = S // ST      # 4 s-tiles
DK = D // 128     # 16 contraction chunks
OC = HPC * DH     # 256 head-output columns per core

SWAP_MASK = [i ^ 1 for i in range(32)]   # partition p <-> p^1, per quadrant

_CACHE = {}
LAST_RESULT = None


def _build_program():
    nc = bacc.Bacc("TRN2", target_bir_lowering=False, debug=False,
                   num_devices=N_CORES)

    def din(name, shape, dt):
        return nc.dram_tensor(name, shape, dt, kind="ExternalInput")

    # host-prepacked SBUF layouts: one wide DMA per resident tile
    xsb_d = din("xsb", [128, NT * DK * ST], BF)    # [p][t][ko][s]
    wq_d = din("wqsb", [128, DK * OC], BF)         # [p][ko][m]
    wkv_d = din("wkvsb", [128, DK * 128], BF)      # [p][ko][v(64) k(64)]
    wo_d = din("wosb", [128, DK * OC], BF)         # [p][oc][m]
    cosq_d = din("cosq", [128, S], BF)
    sinq_d = din("sinq", [128, S], BF)
    cosk_d = din("cosk", [128, S], BF)    # k tables live in rows 64..127
    sink_d = din("sink", [128, S], BF)
    tri_d = din("tri", [128, 128], BF)
    idn_d = din("ident", [128, 128], BF)

    y_d = nc.dram_tensor("y", [S, OC], F32, kind="ExternalOutput")
    # gather granularity balances the ~13us per-collective latency floor
    # (collectives serialize) against readiness: full-tile gathers for
    # tiles 0/1 (lots of downstream slack), per-head-pair for tiles 2/3
    # so the tail pieces start the moment their pair finishes.
    og_h = [nc.dram_tensor(f"og{t}", [OC, ST], BF) for t in range(2)]
    oga_h = [nc.dram_tensor(f"og_all{t}", [N_CORES * OC, ST], BF,
                            addr_space="Shared") for t in range(2)]
    og_p = [[nc.dram_tensor(f"og{t}_{j}", [128, ST], BF) for j in range(2)]
            for t in range(2, NT)]
    oga_p = [[nc.dram_tensor(f"og_all{t}_{j}", [N_CORES * 128, ST], BF,
                             addr_space="Shared") for j in range(2)]
             for t in range(2, NT)]
    ccw_d = nc.dram_tensor("ccw", [1, 64], BF)
    ccwa_d = nc.dram_tensor("ccw_all", [N_CORES, 64], BF, addr_space="Shared")

    EXP = mybir.ActivationFunctionType.Exp
    RG = [list(range(N_CORES))]

    with tile.TileContext(nc) as tc:
        with ExitStack() as ctx:
            cp = ctx.enter_context(tc.tile_pool(name="const", bufs=1))
            scp = ctx.enter_context(tc.tile_pool(name="scp", bufs=2, space="PSUM"))
            pavp = ctx.enter_context(tc.tile_pool(name="pavp", bufs=1, space="PSUM"))
            prjp = ctx.enter_context(tc.tile_pool(name="prj", bufs=1, space="PSUM"))
            mscp = ctx.enter_context(tc.tile_pool(name="msc", bufs=1, space="PSUM"))
            rawp = ctx.enter_context(tc.tile_pool(name="raw", bufs=4))
            tmpp = ctx.enter_context(tc.tile_pool(name="tmp", bufs=3))
            expp = ctx.enter_context(tc.tile_pool(name="expp", bufs=3))
            nrmp = ctx.enter_context(tc.tile_pool(name="nrm", bufs=4))

            scr_d = nc.dram_tensor("scratch", [128, 8], F32)

            # ---- PE warm-up: keep the HAM activity monitor busy while the
            # input DMAs stream in, so real matmuls start at 2.4 GHz.
            wsrc = cp.tile([128, 512], BF)
            nc.vector.memset(wsrc[:], 0.25)
            # tiny exp primes the ACT table set during warmup (the first
            # real softmax exp would otherwise pay the ~2.7us table load)
            wex = rawp.tile([1, 16], BF, tag="wex")
            nc.scalar.activation(wex[:], wsrc[0:1, 0:16],
                                 mybir.ActivationFunctionType.Exp)
            pw = mscp.tile([128, 512], F32, tag="msc")
            NWARM = 28
            for i in range(NWARM):
                nc.tensor.matmul(pw[:], wsrc[:, 0:128], wsrc[:],
                                 start=(i == 0), stop=(i == NWARM - 1),
                                 skip_group_check=True)
            wout = rawp.tile([128, 8], F32, tag="wout")
            nc.vector.tensor_copy(wout[:], pw[:, 0:8])
            nc.sync.dma_start(scr_d.ap(), wout[:])

            # ---- resident tensors (load order = consumption order) ----
            wkv_sb = cp.tile([128, DK, 128], BF)
            nc.sync.dma_start(wkv_sb[:], wkv_d.ap())
            bigs = [cp.tile([128, DK, ST], BF, name=f"big{i}", tag=f"big{i}")
                    for i in range(NT)]
            for q in range(4):      # quarter-split so proj starts early
                nc.sync.dma_start(bigs[0][:, ds(4 * q, 4), :],
                                  xsb_d[:, ds(4 * q * ST, 4 * ST)])
            cosk = cp.tile([128, S], BF); nc.sync.dma_start(cosk[:], cosk_d.ap())
            sink = cp.tile([128, S], BF); nc.sync.dma_start(sink[:], sink_d.ap())
            tri = cp.tile([128, 128], BF); nc.sync.dma_start(tri[:], tri_d.ap())
            idn = cp.tile([128, 128], BF); nc.sync.dma_start(idn[:], idn_d.ap())
            wq_sb = cp.tile([128, DK, OC], BF)
            nc.sync.dma_start(wq_sb[:], wq_d.ap())
            cosq = cp.tile([128, S], BF); nc.sync.dma_start(cosq[:], cosq_d.ap())
            sinq = cp.tile([128, S], BF); nc.sync.dma_start(sinq[:], sinq_d.ap())
            for t in range(1, NT):
                nc.sync.dma_start(bigs[t][:], xsb_d[:, ds(t * DK * ST, DK * ST)])
            wo_sb = cp.tile([128, DK, OC], BF)
            nc.sync.dma_start(wo_sb[:], wo_d.ap())

            qT = cp.tile([128, 2, S], BF)
            kT2 = cp.tile([128, S], BF)
            vaug = cp.tile([128, DK, DH + 1], BF)
            nc.vector.memset(vaug[:, :, DH:DH + 1], 1.0)

            # ---- phase 1: projections + RoPE, split into filler parts.
            state = {}

            def emit_accum_part(t, j, part, nparts=4):
                X = bigs[t]
                per = DK // nparts
                if part == 0:
                    state[(t, j, "ps")] = prjp.tile([128, ST], F32, tag="proj",
                                                    name=f"ps{t}_{j}")
                ps = state[(t, j, "ps")]
                for d in range(per * part, per * (part + 1)):
                    lhsT = wq_sb[:, d, ds(128 * j, 128)] if j < 2 else wkv_sb[:, d, :]
                    nc.tensor.matmul(ps[:], lhsT, X[:, d, :],
                                     start=(d == 0), stop=(d == DK - 1),
                                     skip_group_check=True)
                if part == nparts - 1:
                    ps = state.pop((t, j, "ps"))
                    raw = rawp.tile([128, ST], BF, tag="raw")
                    nc.vector.tensor_copy(raw[:], ps[:])
                    state[(t, j)] = raw

            def emit_accum(t, j):
                for part in range(4):
                    emit_accum_part(t, j, part)

            def emit_post(t, j):
                raw = state[(t, j)]
                sl = ds(t * ST, ST)
                sw = tmpp.tile([128, ST], BF, tag="sw")
                nc.vector.stream_shuffle(sw[:], raw[:], SWAP_MASK)
                if j < 2:
                    t1 = tmpp.tile([128, ST], BF, tag="tmp")
                    nc.vector.tensor_mul(t1[:], raw[:], cosq[:, sl])
                    t2 = tmpp.tile([128, ST], BF, tag="tmp")
                    nc.vector.tensor_mul(t2[:], sw[:], sinq[:, sl])
                    nc.vector.tensor_add(qT[:, j, sl], t1[:], t2[:])
                    state.pop((t, j))
                else:
                    t1 = tmpp.tile([128, ST], BF, tag="tmp")
                    nc.vector.tensor_mul(t1[64:128], raw[64:128], cosk[64:128, sl])
                    t2 = tmpp.tile([128, ST], BF, tag="tmp")
                    nc.vector.tensor_mul(t2[64:128], sw[64:128], sink[64:128, sl])
                    nc.vector.tensor_add(kT2[64:128, sl], t1[64:128], t2[64:128])

            def emit_post_pe(t, j):
                # PE fixups for the kv projection: duplicate rotated k to
                # partitions 0..63, transpose v into [sk, dh] layout.
                raw = state.pop((t, j))
                sl = ds(t * ST, ST)
                psd = mscp.tile([64, ST], F32, tag="msc")
                nc.tensor.matmul(psd[:], idn[64:128, 64:128], kT2[64:128, sl],
                                 start=True, stop=True)
                nc.vector.tensor_copy(kT2[0:64, sl], psd[:])
                for j4 in range(4):
                    pv = mscp.tile([128, DH], BF, tag="msc")
                    nc.tensor.transpose(pv[:], raw[0:64, ds(128 * j4, 128)],
                                        idn[0:64, 0:64])
                    nc.vector.tensor_copy(vaug[:, 4 * t + j4, 0:DH], pv[:])

            # ---- phase 3: one 128-query chunk of the output projection,
            # split into filler parts accumulating in the msc or prj bank.
            def emit_wo_part(qt, sb, part, alt=0, nparts=4):
                X = bigs[qt]
                pool, tag = ((mscp, "msc"), (prjp, "proj"))[alt % 2]
                key = (qt, sb, "wo")
                if part == 0:
                    state[key] = pool.tile([128, OC], F32, tag=tag,
                                           name=f"wo{qt}_{sb}")
                py = state[key]
                per = DK // nparts
                for oc in range(per * part, per * (part + 1)):
                    nc.tensor.matmul(py[:], X[:, oc, ds(128 * sb, 128)],
                                     wo_sb[:, oc, :],
                                     start=(oc == 0), stop=(oc == DK - 1),
                                     skip_group_check=True)
                if part == nparts - 1:
                    py = state.pop(key)
                    ysb = nrmp.tile([128, OC], F32, tag="yo")
                    nc.vector.tensor_copy(ysb[:], py[:])
                    nc.scalar.dma_start(y_d[ds(qt * ST + sb * 128, 128), :],
                                        ysb[:])

            def emit_wo_chunk(qt, sb, alt=0):
                for part in range(4):
                    emit_wo_part(qt, sb, part, alt=alt)

            # ---- filler queue: independent PE work woven between attention
            # chunks.  Each entry is (cost_ns, closure).
            fillq = []

            def fill(cost, fn):
                fillq.append((cost, fn))

            def pop_fill(budget_ns):
                spent = 0.0
                while fillq and fillq[0][0] <= budget_ns - spent:
                    cost, fn = fillq.pop(0)
                    fn()
                    spent += cost
                return spent

            def drain_fill():
                while fillq:
                    _, fn = fillq.pop(0)
                    fn()

            # ---- attention on s-tile t, head pair j (heads 2j, 2j+1).
            # Per key chunk: two row-tiled score matmuls (concurrent), one
            # ACT exp over both heads, two attn@v accumulation matmuls.
            # ACT is the per-chunk critical path in late tiles, so filler
            # work is popped between chunks to keep the PE dense.
            ACT_NS = {}  # chunk width -> exp ns

            def emit_attn_pair(t, j):
                sl = ds(t * ST, ST)
                nchunk = 4 * t + 4
                pavs = [pavp.tile([DH + 1, ST], F32, tag=f"pav{e}",
                                  name=f"pav{t}_{j}_{e}")
                        for e in (0, 1)]

                def sc_chunk(kc):
                    sc = scp.tile([128, 2, ST], F32, tag="sc")
                    c = kc - 4 * t
                    for e in (0, 1):
                        po = 64 * e
                        if c < 0:
                            nc.tensor.matmul(sc[:, e, :],
                                             kT2[po:po + 64, ds(128 * kc, 128)],
                                             qT[po:po + 64, j, sl],
                                             start=True, stop=True)
                        else:
                            w = ST - 128 * c
                            nc.tensor.matmul(sc[:, e, ds(128 * c, w)],
                                             kT2[po:po + 64, ds(128 * kc, 128)],
                                             qT[po:po + 64, j,
                                                ds(t * ST + 128 * c, w)],
                                             start=True, stop=True)
                    return sc

                def exp_chunk(kc, sc):
                    et = expp.tile([128, 2, ST], BF, tag="et")
                    c = kc - 4 * t
                    if c < 0:
                        nc.scalar.activation(et[:, :, :], sc[:, :, :], EXP)
                    else:
                        w = ST - 128 * c
                        nc.scalar.activation(et[:, :, ds(128 * c, w)],
                                             sc[:, :, ds(128 * c, w)], EXP)
                        for e in (0, 1):
                            nc.vector.tensor_mul(et[:, e, ds(128 * c, 128)],
                                                 et[:, e, ds(128 * c, 128)],
                                                 tri[:])
                    return et

                def av_chunk(kc, et):
                    c = kc - 4 * t
                    first = (kc == 0)
                    last = (kc == nchunk - 1)
                    for e in (0, 1):
                        if c < 0:
                            nc.tensor.matmul(pavs[e][:, :], vaug[:, kc, :],
                                             et[:, e, :],
                                             start=first, stop=last,
                                             skip_group_check=True)
                        else:
                            w = ST - 128 * c
                            nc.tensor.matmul(pavs[e][:, ds(128 * c, w)],
                                             vaug[:, kc, :],
                                             et[:, e, ds(128 * c, w)],
                                             start=first, stop=last,
                                             skip_group_check=True)

                prev = None
                debt = 0.0
                for kc in range(nchunk):
                    sc = sc_chunk(kc)
                    if prev is not None:
                        av_chunk(kc - 1, prev)
                    prev = exp_chunk(kc, sc)
                    c = kc - 4 * t
                    w = ST if c < 0 else ST - 128 * c
                    # ACT exp time minus PE chunk time (PE modeled at the
                    # GPIO-throttled ~2.0 GHz effective clock)
                    debt += (2 * w + 352) / 1.2 - 3 * w / 2.0
                    debt -= pop_fill(debt)
                av_chunk(nchunk - 1, prev)

                # normalization: evacuate the accumulators to SBUF first
                # (frees both PSUM banks for the next pair's attn@v), then
                # reciprocal of the ones-column denominator, broadcast to
                # the 64 head dims, divide, store to og.
                ev = []
                for e in (0, 1):
                    # dn copy remaps the denominator row to partition 0
                    # (reciprocal_approx_fast can't remap partition bases)
                    dn = nrmp.tile([1, ST], F32, tag="dn")
                    nc.vector.tensor_copy(dn[:], pavs[e][DH:DH + 1, :])
                    pvs = nrmp.tile([DH, ST], F32, tag="pvs")
                    nc.vector.tensor_copy(pvs[:], pavs[e][0:DH, :])
                    ev.append((dn, pvs))
                for e in (0, 1):
                    h = 2 * j + e
                    dn, pvs = ev[e]
                    rec = nrmp.tile([1, ST], F32, tag="rec")
                    nc.vector.reciprocal_approx_fast(out=rec[:], in_=dn[:])
                    rep = nrmp.tile([DH, ST], F32, tag="rep")
                    nc.gpsimd.partition_broadcast(rep[:], rec[:])
                    on = nrmp.tile([DH, ST], BF, tag="on")
                    nc.vector.tensor_mul(on[:], pvs[:], rep[:])
                    if t < 2:
                        nc.gpsimd.dma_start(og_h[t][ds(DH * h, DH), :], on[:])
                    else:
                        nc.gpsimd.dma_start(og_p[t - 2][j][ds(DH * e, DH), :],
                                            on[:])

            # ---- prologue: proj(t0), kv first so attention can start sooner
            emit_accum(0, 2)
            emit_accum(0, 0)
            emit_post(0, 2)
            emit_post_pe(0, 2)
            emit_accum(0, 1)
            emit_post(0, 0)
            emit_post(0, 1)

            # ---- main loop over s-tiles.  Filler distribution: next tile's
            # projections weave into the current tile's pairs (at t=0 only
            # into pair 1 -- the bigs[1] input DMA hasn't landed earlier);
            # wo chunks for gathered tiles weave into t=2 pair 1 and t=3.
            for t in range(NT):
                nxt = t + 1 < NT
                for j in range(2):
                    # t=0: no proj fillers -- the bigs[1] input DMA lands
                    # only around the end of tile 0's attention; blocking
                    # filler matmuls would poison the attention pipeline.
                    if nxt and t > 0:
                        if j == 0:
                            for p in range(8):
                                fill(430, lambda t=t, p=p:
                                     emit_accum_part(t + 1, 2, p, nparts=8))
                            for p in range(8):
                                fill(430, lambda t=t, p=p:
                                     emit_accum_part(t + 1, 0, p, nparts=8))
                        else:
                            for p in range(8):
                                fill(430, lambda t=t, p=p:
                                     emit_accum_part(t + 1, 1, p, nparts=8))
                    if t == 2 and j == 1:
                        # wo for gathered tile 0 (its reload landed long ago)
                        for sb in range(2):
                            for p in range(4):
                                fill(450, lambda sb=sb, p=p:
                                     emit_wo_part(0, sb, p, alt=0))
                    if t == 3 and j == 0:
                        # wo chunks for gathered tile 0 only (its reload
                        # landed ~50us ago).  Nothing reload-recent in tile
                        # 3 -- a filler whose weights wait on a fresh gather
                        # reload head-of-line-stalls the PE FIFO, poisoning
                        # the last pairs' attention and the final gathers.
                        wol = [(0, 2), (0, 3)]
                        for i, (qt, sb) in enumerate(wol):
                            for p in range(4):
                                fill(450, lambda qt=qt, sb=sb, p=p, i=i:
                                     emit_wo_part(qt, sb, p, alt=i))
                    emit_attn_pair(t, j)
                    # gather this tile's head outputs; the gathered og
                    # replaces xT in bigs[t] for the wo phase.  Tiles 2/3
                    # gather per pair (pair j's heads land in o-chunks 2c+j).
                    if t >= 2:
                        nc.gpsimd.collective_compute(
                            "AllGather", mybir.AluOpType.bypass,
                            replica_groups=RG,
                            ins=[og_p[t - 2][j].ap()],
                            outs=[oga_p[t - 2][j].ap()])
                        for g in range(2):
                            nc.sync.dma_start(
                                bigs[t][:, ds(8 * g + j, 4, 2), :],
                                oga_p[t - 2][j].ap().rearrange(
                                    "(ko p) m -> p ko m", p=128)[:, ds(4 * g, 4), :])
                    elif j == 1:
                        nc.gpsimd.collective_compute(
                            "AllGather", mybir.AluOpType.bypass,
                            replica_groups=RG,
                            ins=[og_h[t].ap()], outs=[oga_h[t].ap()])
                        for g in range(4):
                            nc.sync.dma_start(
                                bigs[t][:, ds(4 * g, 4), :],
                                oga_h[t].ap().rearrange(
                                    "(ko p) m -> p ko m", p=128)[:, ds(4 * g, 4), :])
                    if nxt and t > 0 and j == 0:
                        drain_fill()
                        emit_post(t + 1, 2)
                        fill(1070, lambda t=t: emit_post_pe(t + 1, 2))
                    if nxt and j == 1:
                        drain_fill()
                        if t == 0:
                            # tile-1 projections emitted whole once the
                            # bigs[1] input DMA has landed
                            emit_accum(1, 2)
                            emit_accum(1, 0)
                            emit_post(1, 2)
                            emit_post_pe(1, 2)
                            emit_accum(1, 1)
                        emit_post(t + 1, 0)
                        emit_post(t + 1, 1)
                    if t == 3:
                        drain_fill()

            # ---- wo for the rest of gathered tile 2, then the last s-tile:
            # even o-chunks (first head pair, landed early) accumulate into
            # partial sums while the second gather is in flight; odd chunks
            # finish after it lands.
            for qt in (1, 2):
                for sb in range(4):
                    emit_wo_chunk(qt, sb, alt=sb)
            yev = []
            for sb in range(4):
                pool, tag = ((mscp, "msc"), (prjp, "proj"))[sb % 2]
                py = pool.tile([128, OC], F32, tag=tag)
                for i, oc in enumerate(range(0, DK, 2)):
                    nc.tensor.matmul(py[:], bigs[3][:, oc, ds(128 * sb, 128)],
                                     wo_sb[:, oc, :],
                                     start=(i == 0), stop=(i == 7))
                ye = nrmp.tile([128, OC], F32, tag="ye")
                nc.vector.tensor_copy(ye[:], py[:])
                yev.append(ye)
            # keep the PE's activity monitor busy while waiting for the
            # second half-gather, so the final matmuls run at 2.4 GHz
            wpad = mscp.tile([128, 512], F32, tag="msc")
            NPAD = 24
            for i in range(NPAD):
                nc.tensor.matmul(wpad[:], wsrc[:, 0:128], wsrc[:],
                                 start=(i == 0), stop=(i == NPAD - 1),
                                 skip_group_check=True)
            for sb in range(4):
                pool, tag = ((mscp, "msc"), (prjp, "proj"))[sb % 2]
                py = pool.tile([128, OC], F32, tag=tag)
                for i, oc in enumerate(range(1, DK, 2)):
                    nc.tensor.matmul(py[:], bigs[3][:, oc, ds(128 * sb, 128)],
                                     wo_sb[:, oc, :],
                                     start=(i == 0), stop=(i == 7))
                ysb = nrmp.tile([128, OC], F32, tag="yo2")
                nc.vector.tensor_add(ysb[:], py[:], yev[sb][:])
                nc.scalar.dma_start(y_d[ds(3 * ST + sb * 128, 128), :], ysb[:])

    nc.compile()
    return nc


def _host_prep(x, wq, wk, wv, wo, pos):
    x2 = np.ascontiguousarray(np.asarray(x).reshape(S, D)).astype(BF16)
    # [p][t][ko][s] p-major pack: one wide DMA per s-tile
    xsb = np.ascontiguousarray(
        x2.reshape(NT, ST, DK, 128).transpose(3, 0, 2, 1).reshape(128, -1))

    posf = np.asarray(pos).astype(np.float32)
    fr = (1.0 / (np.float32(THETA) **
                 (np.arange(0, DH, 2, dtype=np.float32) / np.float32(DH))))
    pf = posf[:, None] * fr[None, :]              # [S, 32] f32
    cos = np.cos(pf).astype(np.float32)
    sin = np.sin(pf).astype(np.float32)
    pidx = np.arange(128)
    fi = (pidx % DH) // 2
    sign = np.where(pidx % 2 == 0, np.float32(-1.0), np.float32(1.0))
    cosq = np.ascontiguousarray(cos[:, fi].T)                  # [128, S]
    sinq = np.ascontiguousarray((sin[:, fi] * sign[None, :]).T)
    kscale = np.float32(1.0 / np.sqrt(DH))
    cosk = np.zeros((128, S), np.float32)
    sink = np.zeros((128, S), np.float32)
    cosk[64:128] = cosq[0:64] * kscale
    sink[64:128] = sinq[0:64] * kscale
    cosq = cosq.astype(BF16); sinq = sinq.astype(BF16)
    cosk = cosk.astype(BF16); sink = sink.astype(BF16)

    tri = np.triu(np.ones((128, 128), np.float32)).astype(BF16)
    idn = np.eye(128, dtype=np.float32).astype(BF16)

    woT = np.asarray(wo).T                        # [o, d]
    in_maps = []
    for c in range(N_CORES):
        wq_c = np.asarray(wq)[OC * c: OC * (c + 1), :].astype(BF16)   # [256, D]
        k_c = np.asarray(wk)[DH * c: DH * (c + 1), :].astype(BF16)    # [64, D]
        v_c = np.asarray(wv)[DH * c: DH * (c + 1), :].astype(BF16)
        wkv_c = np.concatenate([v_c, k_c], axis=0)            # [v, k] [128, D]
        wo_c = np.ascontiguousarray(
            woT[:, OC * c: OC * (c + 1)]).astype(BF16)        # [D, OC]
        in_maps.append({
            "xsb": xsb,
            "wqsb": np.ascontiguousarray(
                wq_c.reshape(OC, DK, 128).transpose(2, 1, 0).reshape(128, -1)),
            "wkvsb": np.ascontiguousarray(
                wkv_c.reshape(128, DK, 128).transpose(2, 1, 0).reshape(128, -1)),
            "wosb": np.ascontiguousarray(
                wo_c.reshape(DK, 128, OC).transpose(1, 0, 2).reshape(128, -1)),
            "cosq": cosq, "sinq": sinq, "cosk": cosk, "sink": sink,
            "tri": tri, "ident": idn,
        })
    return in_maps


def kernel(x, pos, wq, wk, wv, wo):
    global LAST_RESULT
    if "nc" not in _CACHE:
        _CACHE["nc"] = _build_program()
    nc = _CACHE["nc"]
    in_maps = _host_prep(x, wq, wk, wv, wo, pos)
    res = run_bass_kernel_spmd(nc, in_maps, core_ids=list(range(N_CORES)))
    LAST_RESULT = res
    y = np.concatenate([res.results[c]["y"] for c in range(N_CORES)], axis=1)
    return y.reshape(1, S, D).astype(np.float32)


# revision 34
# speedup vs baseline: 1.2120x; 1.0221x over previous
"""Tensor-parallel GQA attention block for 8 Trainium2 NeuronCores.

Sharding: 32 q-heads / 8 kv-heads split across 8 cores (4 q-heads + 1
kv-head each).  Each core projects q/k/v from the full x, applies RoPE,
runs causal attention for its heads, then the per-core head outputs are
AllGathered (one gather per 512-query s-tile, so the collectives overlap
attention) and every core computes a distinct 256-column slice of the
final wo projection.  Host concatenates the slices.

Key structure (v2):
  * Attention processes heads in PAIRS.  The even head's k/q live on
    partitions 0..63, the odd head's on 64..127, so the two score
    matmuls of a key chunk run CONCURRENTLY on disjoint PE row-groups
    (tile_position row tiling) -- K=64 contraction no longer wastes
    half the array's issue slots.
  * All big inputs are host-prepacked into SBUF-layout ([128, ...]
    p-major) tensors so each resident tile loads with one wide DMA
    (16-32KB per partition line) at full HBM bandwidth.
  * ACT only runs the softmax exps, one instruction per key chunk
    covering both heads of the pair.  In late s-tiles ACT is the
    per-chunk critical path, so independent PE work (next tile's
    projections, wo chunks, kv fixups) is woven between chunks via a
    debt-driven filler queue to keep the PE dense.
  * Softmax is unnormalized; the denominator comes out of the attn@v
    matmul via a ones column and is divided out on DVE.
  * A tiny dummy AllGather during warmup absorbs the ~10us first-CC
    setup cost so the real per-tile gathers start promptly.

PSUM (8 banks): score pairs [128,2,512]f32 x2bufs = 4, attn@v accum
[65,512]f32 x2 (even/odd head) = 2, projection accum x1 = 1,
warmup/fixups/wo x1 = 1.
"""

import sys

sys.path.insert(0, "/opt/trn_rl_repo")

import numpy as np
import ml_dtypes
from contextlib import ExitStack

import concourse.bass as bass
import concourse.tile as tile
from concourse import bacc, mybir
from concourse.bass import ds
from concourse.bass_utils import run_bass_kernel_spmd

BF16 = ml_dtypes.bfloat16
F32 = mybir.dt.float32
BF = mybir.dt.bfloat16

N_CORES = 8
S = 2048          # sequence length
D = 2048          # model dim
DH = 64           # head dim
HPC = 4           # q heads per core
THETA = 10000.0
ST = 512          # s-tile (free dim) size
NT = S // ST      # 4 s-tiles
DK = D // 128     # 16 contraction chunks
OC = HPC * DH     # 256 head-output columns per core

SWAP_MASK = [i ^ 1 for i in range(32)]   # partition p <-> p^1, per quadrant

_CACHE = {}
LAST_RESULT = None


def _build_program():
    nc = bacc.Bacc("TRN2", target_bir_lowering=False, debug=False,
                   num_devices=N_CORES)

    def din(name, shape, dt):
        return nc.dram_tensor(name, shape, dt, kind="ExternalInput")

    # host-prepacked SBUF layouts: one wide DMA per resident tile
    xsb_d = din("xsb", [128, NT * DK * ST], BF)    # [p][t][ko][s]
    wq_d = din("wqsb", [128, DK * OC], BF)         # [p][ko][m]
    wkv_d = din("wkvsb", [128, DK * 128], BF)      # [p][ko][v(64) k(64)]
    wo_d = din("wosb", [128, DK * OC], BF)         # [p][oc][m]
    cosq_d = din("cosq", [128, S], BF)
    sinq_d = din("sinq", [128, S], BF)
    cosk_d = din("cosk", [128, S], BF)    # k tables live in rows 64..127
    sink_d = din("sink", [128, S], BF)
    tri_d = din("tri", [128, 128], BF)
    idn_d = din("ident", [128, 128], BF)

    y_d = nc.dram_tensor("y", [S, OC], F32, kind="ExternalOutput")
    # gather granularity balances the ~13us per-collective latency floor
    # (collectives serialize) against readiness: full-tile gathers for
    # tiles 0/1 (lots of downstream slack), per-head-pair for tiles 2/3
    # so the tail pieces start the moment their pair finishes.
    og_h = [nc.dram_tensor(f"og{t}", [OC, ST], BF) for t in range(2)]
    oga_h = [nc.dram_tensor(f"og_all{t}", [N_CORES * OC, ST], BF,
                            addr_space="Shared") for t in range(2)]
    og_p = [[nc.dram_tensor(f"og{t}_{j}", [128, ST], BF) for j in range(2)]
            for t in range(2, NT)]
    oga_p = [[nc.dram_tensor(f"og_all{t}_{j}", [N_CORES * 128, ST], BF,
                             addr_space="Shared") for j in range(2)]
             for t in range(2, NT)]
    ccw_d = nc.dram_tensor("ccw", [1, 64], BF)
    ccwa_d = nc.dram_tensor("ccw_all", [N_CORES, 64], BF, addr_space="Shared")

    EXP = mybir.ActivationFunctionType.Exp
    RG = [list(range(N_CORES))]

    with tile.TileContext(nc) as tc:
        with ExitStack() as ctx:
            cp = ctx.enter_context(tc.tile_pool(name="const", bufs=1))
            scp = ctx.enter_context(tc.tile_pool(name="scp", bufs=2, space="PSUM"))
            pavp = ctx.enter_context(tc.tile_pool(name="pavp", bufs=1, space="PSUM"))
            prjp = ctx.enter_context(tc.tile_pool(name="prj", bufs=1, space="PSUM"))
            mscp = ctx.enter_context(tc.tile_pool(name="msc", bufs=1, space="PSUM"))
            rawp = ctx.enter_context(tc.tile_pool(name="raw", bufs=4))
            tmpp = ctx.enter_context(tc.tile_pool(name="tmp", bufs=3))
            expp = ctx.enter_context(tc.tile_pool(name="expp", bufs=3))
            nrmp = ctx.enter_context(tc.tile_pool(name="nrm", bufs=4))

            scr_d = nc.dram_tensor("scratch", [128, 8], F32)

            # ---- PE warm-up: keep the HAM activity monitor busy while the
            # input DMAs stream in, so real matmuls start at 2.4 GHz.
            wsrc = cp.tile([128, 512], BF)
            nc.vector.memset(wsrc[:], 0.25)
            # tiny exp primes the ACT table set during warmup (the first
            # real softmax exp would otherwise pay the ~2.7us table load)
            wex = rawp.tile([1, 16], BF, tag="wex")
            nc.scalar.activation(wex[:], wsrc[0:1, 0:16],
                                 mybir.ActivationFunctionType.Exp)
            pw = mscp.tile([128, 512], F32, tag="msc")
            NWARM = 28
            for i in range(NWARM):
                nc.tensor.matmul(pw[:], wsrc[:, 0:128], wsrc[:],
                                 start=(i == 0), stop=(i == NWARM - 1),
                                 skip_group_check=True)
            wout = rawp.tile([128, 8], F32, tag="wout")
            nc.vector.tensor_copy(wout[:], pw[:, 0:8])
            nc.sync.dma_start(scr_d.ap(), wout[:])

            # ---- resident tensors (load order = consumption order) ----
            wkv_sb = cp.tile([128, DK, 128], BF)
            nc.sync.dma_start(wkv_sb[:], wkv_d.ap())
            bigs = [cp.tile([128, DK, ST], BF, name=f"big{i}", tag=f"big{i}")
                    for i in range(NT)]
            for q in range(4):      # quarter-split so proj starts early
                nc.sync.dma_start(bigs[0][:, ds(4 * q, 4), :],
                                  xsb_d[:, ds(4 * q * ST, 4 * ST)])
            cosk = cp.tile([128, S], BF); nc.sync.dma_start(cosk[:], cosk_d.ap())
            sink = cp.tile([128, S], BF); nc.sync.dma_start(sink[:], sink_d.ap())
            tri = cp.tile([128, 128], BF); nc.sync.dma_start(tri[:], tri_d.ap())
            idn = cp.tile([128, 128], BF); nc.sync.dma_start(idn[:], idn_d.ap())
            wq_sb = cp.tile([128, DK, OC], BF)
            nc.sync.dma_start(wq_sb[:], wq_d.ap())
            cosq = cp.tile([128, S], BF); nc.sync.dma_start(cosq[:], cosq_d.ap())
            sinq = cp.tile([128, S], BF); nc.sync.dma_start(sinq[:], sinq_d.ap())
            for t in range(1, NT):
                nc.sync.dma_start(bigs[t][:], xsb_d[:, ds(t * DK * ST, DK * ST)])
            wo_sb = cp.tile([128, DK, OC], BF)
            nc.sync.dma_start(wo_sb[:], wo_d.ap())

            qT = cp.tile([128, 2, S], BF)
            kT2 = cp.tile([128, S], BF)
            vaug = cp.tile([128, DK, DH + 1], BF)
            nc.vector.memset(vaug[:, :, DH:DH + 1], 1.0)

            # ---- phase 1: projections + RoPE, split into filler parts.
            state = {}

            def emit_accum_part(t, j, part, nparts=4):
                X = bigs[t]
                per = DK // nparts
                if part == 0:
                    state[(t, j, "ps")] = prjp.tile([128, ST], F32, tag="proj",
                                                    name=f"ps{t}_{j}")
                ps = state[(t, j, "ps")]
                for d in range(per * part, per * (part + 1)):
                    lhsT = wq_sb[:, d, ds(128 * j, 128)] if j < 2 else wkv_sb[:, d, :]
                    nc.tensor.matmul(ps[:], lhsT, X[:, d, :],
                                     start=(d == 0), stop=(d == DK - 1),
                                     skip_group_check=True)
                if part == nparts - 1:
                    ps = state.pop((t, j, "ps"))
                    raw = rawp.tile([128, ST], BF, tag="raw")
                    nc.vector.tensor_copy(raw[:], ps[:])
                    state[(t, j)] = raw

            def emit_accum(t, j):
                for part in range(4):
                    emit_accum_part(t, j, part)

            def emit_post(t, j):
                raw = state[(t, j)]
                sl = ds(t * ST, ST)
                sw = tmpp.tile([128, ST], BF, tag="sw")
                nc.vector.stream_shuffle(sw[:], raw[:], SWAP_MASK)
                if j < 2:
                    t1 = tmpp.tile([128, ST], BF, tag="tmp")
                    nc.vector.tensor_mul(t1[:], raw[:], cosq[:, sl])
                    t2 = tmpp.tile([128, ST], BF, tag="tmp")
                    nc.vector.tensor_mul(t2[:], sw[:], sinq[:, sl])
                    nc.vector.tensor_add(qT[:, j, sl], t1[:], t2[:])
                    state.pop((t, j))
                else:
                    t1 = tmpp.tile([128, ST], BF, tag="tmp")
                    nc.vector.tensor_mul(t1[64:128], raw[64:128], cosk[64:128, sl])
                    t2 = tmpp.tile([128, ST], BF, tag="tmp")
                    nc.vector.tensor_mul(t2[64:128], sw[64:128], sink[64:128, sl])
                    nc.vector.tensor_add(kT2[64:128, sl], t1[64:128], t2[64:128])

            def emit_post_pe(t, j):
                # PE fixups for the kv projection: duplicate rotated k to
                # partitions 0..63, transpose v into [sk, dh] layout.
                raw = state.pop((t, j))
                sl = ds(t * ST, ST)
                psd = mscp.tile([64, ST], F32, tag="msc")
                nc.tensor.matmul(psd[:], idn[64:128, 64:128], kT2[64:128, sl],
                                 start=True, stop=True)
                nc.vector.tensor_copy(kT2[0:64, sl], psd[:])
                for j4 in range(4):
                    pv = mscp.tile([128, DH], BF, tag="msc")
                    nc.tensor.transpose(pv[:], raw[0:64, ds(128 * j4, 128)],
                                        idn[0:64, 0:64])
                    nc.vector.tensor_copy(vaug[:, 4 * t + j4, 0:DH], pv[:])

            # ---- phase 3: one 128-query chunk of the output projection,
            # split into filler parts accumulating in the msc or prj bank.
            def emit_wo_part(qt, sb, part, alt=0, nparts=4):
                X = bigs[qt]
                pool, tag = ((mscp, "msc"), (prjp, "proj"))[alt % 2]
                key = (qt, sb, "wo")
                if part == 0:
                    state[key] = pool.tile([128, OC], F32, tag=tag,
                                           name=f"wo{qt}_{sb}")
                py = state[key]
                per = DK // nparts
                for oc in range(per * part, per * (part + 1)):
                    nc.tensor.matmul(py[:], X[:, oc, ds(128 * sb, 128)],
                                     wo_sb[:, oc, :],
                                     start=(oc == 0), stop=(oc == DK - 1),
                                     skip_group_check=True)
                if part == nparts - 1:
                    py = state.pop(key)
                    ysb = nrmp.tile([128, OC], F32, tag="yo")
                    nc.vector.tensor_copy(ysb[:], py[:])
                    nc.scalar.dma_start(y_d[ds(qt * ST + sb * 128, 128), :],
                                        ysb[:])

            def emit_wo_chunk(qt, sb, alt=0):
                for part in range(4):
                    emit_wo_part(qt, sb, part, alt=alt)

            # ---- filler queue: independent PE work woven between attention
            # chunks.  Each entry is (cost_ns, closure).
            fillq = []

            def fill(cost, fn):
                fillq.append((cost, fn))

            def pop_fill(budget_ns):
                spent = 0.0
                while fillq and fillq[0][0] <= budget_ns - spent:
                    cost, fn = fillq.pop(0)
                    fn()
                    spent += cost
                return spent

            def drain_fill():
                while fillq:
                    _, fn = fillq.pop(0)
                    fn()

            # ---- attention on s-tile t, head pair j (heads 2j, 2j+1).
            # Per key chunk: two row-tiled score matmuls (concurrent), one
            # ACT exp over both heads, two attn@v accumulation matmuls.
            # ACT is the per-chunk critical path in late tiles, so filler
            # work is popped between chunks to keep the PE dense.
            ACT_NS = {}  # chunk width -> exp ns

            def emit_attn_pair(t, j):
                sl = ds(t * ST, ST)
                nchunk = 4 * t + 4
                pavs = [pavp.tile([DH + 1, ST], F32, tag=f"pav{e}",
                                  name=f"pav{t}_{j}_{e}")
                        for e in (0, 1)]

                def sc_chunk(kc):
                    sc = scp.tile([128, 2, ST], F32, tag="sc")
                    c = kc - 4 * t
                    for e in (0, 1):
                        po = 64 * e
                        if c < 0:
                            nc.tensor.matmul(sc[:, e, :],
                                             kT2[po:po + 64, ds(128 * kc, 128)],
                                             qT[po:po + 64, j, sl],
                                             start=True, stop=True)
                        else:
                            w = ST - 128 * c
                            nc.tensor.matmul(sc[:, e, ds(128 * c, w)],
                                             kT2[po:po + 64, ds(128 * kc, 128)],
                                             qT[po:po + 64, j,
                                                ds(t * ST + 128 * c, w)],
                                             start=True, stop=True)
                    return sc

                def exp_chunk(kc, sc):
                    et = expp.tile([128, 2, ST], BF, tag="et")
                    c = kc - 4 * t
                    if c < 0:
                        nc.scalar.activation(et[:, :, :], sc[:, :, :], EXP)
                    else:
                        w = ST - 128 * c
                        nc.scalar.activation(et[:, :, ds(128 * c, w)],
                                             sc[:, :, ds(128 * c, w)], EXP)
                        for e in (0, 1):
                            nc.vector.tensor_mul(et[:, e, ds(128 * c, 128)],
                                                 et[:, e, ds(128 * c, 128)],
                                                 tri[:])
                    return et

                def av_chunk(kc, et):
                    c = kc - 4 * t
                    first = (kc == 0)
                    last = (kc == nchunk - 1)
                    for e in (0, 1):
                        if c < 0:
                            nc.tensor.matmul(pavs[e][:, :], vaug[:, kc, :],
                                             et[:, e, :],
                                             start=first, stop=last,
                                             skip_group_check=True)
                        else:
                            w = ST - 128 * c
                            nc.tensor.matmul(pavs[e][:, ds(128 * c, w)],
                                             vaug[:, kc, :],
                                             et[:, e, ds(128 * c, w)],
                                             start=first, stop=last,
                                             skip_group_check=True)

                prev = None
                debt = 0.0
                for kc in range(nchunk):
                    sc = sc_chunk(kc)
                    if prev is not None:
                        av_chunk(kc - 1, prev)
                    prev = exp_chunk(kc, sc)
                    c = kc - 4 * t
                    w = ST if c < 0 else ST - 128 * c
                    # ACT exp time minus PE chunk time (PE modeled at the
                    # GPIO-throttled ~2.0 GHz effective clock)
                    debt += (2 * w + 352) / 1.2 - 3 * w / 2.0
                    debt -= pop_fill(debt)
                av_chunk(nchunk - 1, prev)

                # normalization: evacuate the accumulators to SBUF first
                # (frees both PSUM banks for the next pair's attn@v), then
                # reciprocal of the ones-column denominator, broadcast to
                # the 64 head dims, divide, store to og.
                ev = []
                for e in (0, 1):
                    # dn copy remaps the denominator row to partition 0
                    # (reciprocal_approx_fast can't remap partition bases)
                    dn = nrmp.tile([1, ST], F32, tag="dn")
                    nc.vector.tensor_copy(dn[:], pavs[e][DH:DH + 1, :])
                    pvs = nrmp.tile([DH, ST], F32, tag="pvs")
                    nc.vector.tensor_copy(pvs[:], pavs[e][0:DH, :])
                    ev.append((dn, pvs))
                for e in (0, 1):
                    h = 2 * j + e
                    dn, pvs = ev[e]
                    rec = nrmp.tile([1, ST], F32, tag="rec")
                    nc.vector.reciprocal_approx_fast(out=rec[:], in_=dn[:])
                    rep = nrmp.tile([DH, ST], F32, tag="rep")
                    nc.gpsimd.partition_broadcast(rep[:], rec[:])
                    on = nrmp.tile([DH, ST], BF, tag="on")
                    nc.vector.tensor_mul(on[:], pvs[:], rep[:])
                    if t < 2:
                        nc.gpsimd.dma_start(og_h[t][ds(DH * h, DH), :], on[:])
                    else:
                        nc.gpsimd.dma_start(og_p[t - 2][j][ds(DH * e, DH), :],
                                            on[:])

            # ---- prologue: proj(t0), kv first so attention can start sooner
            emit_accum(0, 2)
            emit_accum(0, 0)
            emit_post(0, 2)
            emit_post_pe(0, 2)
            emit_accum(0, 1)
            emit_post(0, 0)
            emit_post(0, 1)

            # ---- main loop over s-tiles.  Filler distribution: next tile's
            # projections weave into the current tile's pairs (at t=0 only
            # into pair 1 -- the bigs[1] input DMA hasn't landed earlier);
            # wo chunks for gathered tiles weave into t=2 pair 1 and t=3.
            for t in range(NT):
                nxt = t + 1 < NT
                for j in range(2):
                    # t=0: no proj fillers -- the bigs[1] input DMA lands
                    # only around the end of tile 0's attention; blocking
                    # filler matmuls would poison the attention pipeline.
                    if nxt and t > 0:
                        if j == 0:
                            for p in range(8):
                                fill(430, lambda t=t, p=p:
                                     emit_accum_part(t + 1, 2, p, nparts=8))
                            for p in range(8):
                                fill(430, lambda t=t, p=p:
                                     emit_accum_part(t + 1, 0, p, nparts=8))
                        else:
                            for p in range(8):
                                fill(430, lambda t=t, p=p:
                                     emit_accum_part(t + 1, 1, p, nparts=8))
                    if t == 2 and j == 1:
                        # wo for gathered tile 0 (its reload landed long ago)
                        for sb in range(2):
                            for p in range(4):
                                fill(450, lambda sb=sb, p=p:
                                     emit_wo_part(0, sb, p, alt=0))
                    if t == 3 and j == 0:
                        # wo chunks for gathered tiles 0/1 as filler.  None
                        # in pair (3,1): a filler waiting on the tile-2
                        # reload there would poison the last pair's
                        # attention and delay the final gather.
                        wol = [(0, 2), (0, 3), (1, 0), (1, 1), (1, 2), (1, 3)]
                        for i, (qt, sb) in enumerate(wol):
                            for p in range(4):
                                fill(450, lambda qt=qt, sb=sb, p=p, i=i:
                                     emit_wo_part(qt, sb, p, alt=i))
                    emit_attn_pair(t, j)
                    # gather this tile's head outputs; the gathered og
                    # replaces xT in bigs[t] for the wo phase.  Tiles 2/3
                    # gather per pair (pair j's heads land in o-chunks 2c+j).
                    if t >= 2:
                        nc.gpsimd.collective_compute(
                            "AllGather", mybir.AluOpType.bypass,
                            replica_groups=RG,
                            ins=[og_p[t - 2][j].ap()],
                            outs=[oga_p[t - 2][j].ap()])
                        for g in range(2):
                            nc.sync.dma_start(
                                bigs[t][:, ds(8 * g + j, 4, 2), :],
                                oga_p[t - 2][j].ap().rearrange(
                                    "(ko p) m -> p ko m", p=128)[:, ds(4 * g, 4), :])
                    elif j == 1:
                        nc.gpsimd.collective_compute(
                            "AllGather", mybir.AluOpType.bypass,
                            replica_groups=RG,
                            ins=[og_h[t].ap()], outs=[oga_h[t].ap()])
                        for g in range(4):
                            nc.sync.dma_start(
                                bigs[t][:, ds(4 * g, 4), :],
                                oga_h[t].ap().rearrange(
                                    "(ko p) m -> p ko m", p=128)[:, ds(4 * g, 4), :])
                    if nxt and t > 0 and j == 0:
                        drain_fill()
                        emit_post(t + 1, 2)
                        fill(1070, lambda t=t: emit_post_pe(t + 1, 2))
                    if nxt and j == 1:
                        drain_fill()
                        if t == 0:
                            # tile-1 projections emitted whole once the
                            # bigs[1] input DMA has landed
                            emit_accum(1, 2)
                            emit_accum(1, 0)
                            emit_post(1, 2)
                            emit_post_pe(1, 2)
                            emit_accum(1, 1)
                        emit_post(t + 1, 0)
                        emit_post(t + 1, 1)
                    if t == 3:
                        drain_fill()

            # ---- wo for the rest of gathered tile 2, then the last s-tile:
            # even o-chunks (first head pair, landed early) accumulate into
            # partial sums while the second gather is in flight; odd chunks
            # finish after it lands.
            for sb in range(4):
                emit_wo_chunk(2, sb, alt=sb)
            yev = []
            for sb in range(4):
                pool, tag = ((mscp, "msc"), (prjp, "proj"))[sb % 2]
                py = pool.tile([128, OC], F32, tag=tag)
                for i, oc in enumerate(range(0, DK, 2)):
                    nc.tensor.matmul(py[:], bigs[3][:, oc, ds(128 * sb, 128)],
                                     wo_sb[:, oc, :],
                                     start=(i == 0), stop=(i == 7))
                ye = nrmp.tile([128, OC], F32, tag="ye")
                nc.vector.tensor_copy(ye[:], py[:])
                yev.append(ye)
            # keep the PE's activity monitor busy while waiting for the
            # second half-gather, so the final matmuls run at 2.4 GHz
            wpad = mscp.tile([128, 512], F32, tag="msc")
            NPAD = 24
            for i in range(NPAD):
                nc.tensor.matmul(wpad[:], wsrc[:, 0:128], wsrc[:],
                                 start=(i == 0), stop=(i == NPAD - 1),
                                 skip_group_check=True)
            for sb in range(4):
                pool, tag = ((mscp, "msc"), (prjp, "proj"))[sb % 2]
                py = pool.tile([128, OC], F32, tag=tag)
                for i, oc in enumerate(range(1, DK, 2)):
                    nc.tensor.matmul(py[:], bigs[3][:, oc, ds(128 * sb, 128)],
                                     wo_sb[:, oc, :],
                                     start=(i == 0), stop=(i == 7))
                ysb = nrmp.tile([128, OC], F32, tag="yo2")
                nc.vector.tensor_add(ysb[:], py[:], yev[sb][:])
                nc.scalar.dma_start(y_d[ds(3 * ST + sb * 128, 128), :], ysb[:])

    nc.compile()
    return nc


def _host_prep(x, wq, wk, wv, wo, pos):
    x2 = np.ascontiguousarray(np.asarray(x).reshape(S, D)).astype(BF16)
    # [p][t][ko][s] p-major pack: one wide DMA per s-tile
    xsb = np.ascontiguousarray(
        x2.reshape(NT, ST, DK, 128).transpose(3, 0, 2, 1).reshape(128, -1))

    posf = np.asarray(pos).astype(np.float32)
    fr = (1.0 / (np.float32(THETA) **
                 (np.arange(0, DH, 2, dtype=np.float32) / np.float32(DH))))
    pf = posf[:, None] * fr[None, :]              # [S, 32] f32
    cos = np.cos(pf).astype(np.float32)
    sin = np.sin(pf).astype(np.float32)
    pidx = np.arange(128)
    fi = (pidx % DH) // 2
    sign = np.where(pidx % 2 == 0, np.float32(-1.0), np.float32(1.0))
    cosq = np.ascontiguousarray(cos[:, fi].T)                  # [128, S]
    sinq = np.ascontiguousarray((sin[:, fi] * sign[None, :]).T)
    kscale = np.float32(1.0 / np.sqrt(DH))
    cosk = np.zeros((128, S), np.float32)
    sink = np.zeros((128, S), np.float32)
    cosk[64:128] = cosq[0:64] * kscale
    sink[64:128] = sinq[0:64] * kscale
    cosq = cosq.astype(BF16); sinq = sinq.astype(BF16)
    cosk = cosk.astype(BF16); sink = sink.astype(BF16)

    tri = np.triu(np.ones((128, 128), np.float32)).astype(BF16)
    idn = np.eye(128, dtype=np.float32).astype(BF16)

    woT = np.asarray(wo).T                        # [o, d]
    in_maps = []
    for c in range(N_CORES):
        wq_c = np.asarray(wq)[OC * c: OC * (c + 1), :].astype(BF16)   # [256, D]
        k_c = np.asarray(wk)[DH * c: DH * (c + 1), :].astype(BF16)    # [64, D]
        v_c = np.asarray(wv)[DH * c: DH * (c + 1), :].astype(BF16)
        wkv_c = np.concatenate([v_c, k_c], axis=0)            # [v, k] [128, D]
        wo_c = np.ascontiguousarray(
            woT[:, OC * c: OC * (c + 1)]).astype(BF16)        # [D, OC]
        in_maps.append({
            "xsb": xsb,
            "wqsb": np.ascontiguousarray(
                wq_c.reshape(OC, DK, 128).transpose(2, 1, 0).reshape(128, -1)),
            "wkvsb": np.ascontiguousarray(
                wkv_c.reshape(128, DK, 128).transpose(2, 1, 0).reshape(128, -1)),
            "wosb": np.ascontiguousarray(
                wo_c.reshape(DK, 128, OC).transpose(1, 0, 2).reshape(128, -1)),
            "cosq": cosq, "sinq": sinq, "cosk": cosk, "sink": sink,
            "tri": tri, "ident": idn,
        })
    return in_maps


def kernel(x, pos, wq, wk, wv, wo):
    global LAST_RESULT
    if "nc" not in _CACHE:
        _CACHE["nc"] = _build_program()
    nc = _CACHE["nc"]
    in_maps = _host_prep(x, wq, wk, wv, wo, pos)
    res = run_bass_kernel_spmd(nc, in_maps, core_ids=list(range(N_CORES)))
    LAST_RESULT = res
    y = np.concatenate([res.results[c]["y"] for c in range(N_CORES)], axis=1)
    return y.reshape(1, S, D).astype(np.float32)
